# revision 34
# baseline (speedup 1.0000x reference)
"""NT-Xent loss kernel for Trainium2 (8 NeuronCores, SPMD).

Strategy (v4, symmetric half-slab):
  Host: z = concat(z_i, z_j) [8192, 256] f32; normalize rows
  (eps-clamped), fold temperature (x sqrt(10)), quantize to fp8 e4m3,
  lay out the DoubleRow moving operand X[p, j, t] = q[j, 128t + p]
  (k-tile dim innermost so every slice is byte-contained for the tile
  tracker and ifmap streaming reads contiguous byte pairs), plus an
  m-contiguous copy of the first 1024 columns for the LDWEIGHTS-side.
  Each core gets a rotated copy (roll j by -1024c).

  The 8192^2 sim matrix is symmetric: exp(sim) is computed once per
  unordered pair.  Per core, row-block m computes local column blocks
  [m, m+31] as fp8 DoubleRow matmuls in [128, <=1536] PSUM pieces
  (8 x 512-col chunks per row).  exp + row-sum accumulation runs on
  ACT (Exp, accum_out) and on the DVE via a 3-pass Schraudolph
  bit-trick exp (pass2 on GPSIMD), balanced by a static assignment.
  exp values are also written as fp8e5 into per-pair interleaved
  buffers; one fp8e5 DoubleRow ones-matmul per 512-col chunk
  column-sums BOTH rows of a pair at once, routed via a sliding-window
  one-hot stationary to its own partition slot of one PSUM bank.
  Device outputs: per-row-block row-sum parts [128, 8] and the colsum
  bank [40, 512].

  Host combine: scatter-add colsum slots into the global row-sum
  vector, subtract exp(||q_i||^2) (the unmasked self-similarity the
  device accumulated), add the d=32 "ring" block row sums + read the
  positive sims off its diagonal (one batched [64,128,128] gemm on
  q), then loss = mean(ln(rowsum) - pos).
"""

import sys

sys.path.insert(0, "/opt/trn_rl_repo")

import numpy as np
import ml_dtypes

import concourse.tile as tile
from concourse import bacc, mybir
from concourse.bass_utils import run_bass_kernel_spmd

F32 = mybir.dt.float32
BF16 = mybir.dt.bfloat16
I32 = mybir.dt.int32
FP8E4 = mybir.dt.float8e4
FP8E5 = mybir.dt.float8e5

B = 4096
D = 256
N = 2 * B           # 8192
NCORES = 8
ROWS = N // NCORES  # 1024 rows per core
MB = ROWS // 128    # 8 row-blocks per core
SPAN = 32 * 128     # 4096 cols per row strip (self + 31 colsum blocks)
XCOLS = 5120        # DMA'd columns per core (covers 896 + 4096)
PW = (1536, 1536, 1024)   # piece widths
SQRT10 = float(np.sqrt(10.0))

# Schraudolph-style exp in 3 passes (DVE, GPSIMD, DVE):
#   t = A*x + C keeps t in the binade [2^28, 2^29) for |x| <= 11, so
#   bits(t) = bits(C) + round(A*x/32) exactly; then
#   ebits = (bits(t) + K1) * 32 ~= bits(exp(x)), bias tuned for zero
#   mean multiplicative error on this sim distribution.
EXP_A = float(np.float32(2 ** 23 / np.log(2.0)))
EXP_C = float(np.float32(1.5 * 2 ** 28))
_BINT = 1064870642
EXP_K1 = int(round(_BINT / 32)) - int(np.float32(EXP_C).view(np.int32))

# engine assignment (per row-block): which pieces go to the DVE path
DVE_P1 = (0, 1, 2, 4, 5, 6)
DVE_P2 = (2, 5)


def build_program():
    nc = bacc.Bacc("TRN2", target_bir_lowering=False, debug=False, num_devices=NCORES)
    xq = nc.dram_tensor("xq", [128, XCOLS, 2], FP8E4, kind="ExternalInput")
    # stationary copy of the first 1024 cols in the m-contiguous layout
    # required by the dual-fp8 LDWEIGHTS
    xw = nc.dram_tensor("xw", [128, 2, 1024], FP8E4, kind="ExternalInput")
    oh = nc.dram_tensor("oh", [128, 2, 256], FP8E5, kind="ExternalInput")
    out_se = nc.dram_tensor("sumexp_own", [128, MB * 3], F32, kind="ExternalOutput")
    out_cs = nc.dram_tensor("colsums", [40, 512], F32, kind="ExternalOutput")

    AL = mybir.AluOpType
    AF = mybir.ActivationFunctionType
    DR = mybir.MatmulPerfMode.DoubleRow

    with tile.TileContext(nc) as tc:
        with (
            tc.tile_pool(name="consts", bufs=1) as cpool,
            tc.tile_pool(name="xq", bufs=1) as xpool,
            tc.tile_pool(name="pairs", bufs=1) as prpool,
            tc.tile_pool(name="persist", bufs=1) as ppool,
            tc.tile_pool(name="ps", bufs=2, space="PSUM") as pspool,
        ):
            # xt stripes on the sync queue; the small operands from other
            # engines' DGEs so their transfers don't queue behind xt
            xw_sb = xpool.tile([128, 2, 1024], FP8E4, tag="xw", name="xw")
            nc.sync.dma_start(xw_sb[:], xw[:])
            xt = xpool.tile([128, XCOLS, 2], FP8E4, tag="xt", name="xt")
            for s in range(10):
                nc.sync.dma_start(
                    xt[:, s * 512:(s + 1) * 512, :],
                    xq[:, s * 512:(s + 1) * 512, :])
            oh_sb = cpool.tile([128, 2, 256], FP8E5, tag="oh", name="oh")
            nc.scalar.dma_start(oh_sb[:], oh[:])

            def xop(c0, w):
                return xt[:, c0:c0 + w, :].rearrange("p n t -> p t n")

            # per-pair interleaved e5m2 exp buffers: [p, n, t];
            # t=1 holds row 2i at n = row-rel col, t=0 holds row 2i+1 at
            # n = row-rel col + 128, so (n, t=0/1) sit at the same
            # absolute column 256i + n.
            pairs = prpool.tile([128, 4, 4224, 2], FP8E5, tag="pairs",
                                name="pairs")
            pair_sb = [pairs[:, i] for i in range(4)]

            t_f32 = [ppool.tile([128, 1536], F32, tag=f"t{j}", name=f"t{j}")
                     for j in (0, 1)]
            eb_i32 = [ppool.tile([128, 1536], I32, tag=f"eb{j}", name=f"eb{j}")
                      for j in (0, 1)]
            sexp_parts = ppool.tile([128, MB * 3], F32, tag="sexp")

            cs = pspool.tile([128, 512], F32, tag="cs", name="cs", bufs=1)
            cs_first = [True]

            def cs_matmul(slot, rhs, ncols, last=False, dr=True):
                # sliding-window one-hot: stationary col `slot` of the M=40
                # window is all-ones, so the colsum lands on partition
                # `slot` (and +0 accumulates everywhere else).
                nc.tensor.matmul(
                    cs[0:40, 0:ncols],
                    oh_sb[:, :, 128 - slot:168 - slot] if dr
                    else oh_sb[:, 0, 128 - slot:168 - slot],
                    rhs,
                    start=cs_first[0], stop=last,
                    perf_mode=DR if dr else None,
                )
                cs_first[0] = False

            dve_par = [0]

            def exp_dve(P, W, oslice, sidx):
                j = dve_par[0] = 1 - dve_par[0]
                t, eb = t_f32[j], eb_i32[j]
                nc.vector.tensor_scalar(
                    out=t[:, :W], in0=P[:, :W], scalar1=EXP_A, scalar2=EXP_C,
                    op0=AL.mult, op1=AL.add)
                nc.gpsimd.tensor_scalar(
                    out=eb[:, :W], in0=t[:, :W].bitcast(I32),
                    scalar1=EXP_K1, scalar2=32, op0=AL.add, op1=AL.mult)
                nc.vector.tensor_scalar(
                    out=oslice, in0=eb[:, :W].bitcast(F32),
                    scalar1=1.0, scalar2=0.0, op0=AL.mult, op1=AL.add,
                    accum_out=sexp_parts[:, sidx:sidx + 1])

            def cs_ops_for_pair(i, last_pair):
                """Column-sum matmuls for pair i, emitted one pair later so
                their exp dependencies are long satisfied and the in-order
                PE queue never head-of-line blocks on them."""
                ops = []
                for j in range(8):
                    n0 = 256 + 512 * j
                    nw = min(512, 4096 - n0)
                    ops.append((i * 8 + j,
                                pair_sb[i][:, n0:n0 + nw, :].rearrange(
                                    "p n t -> p t n"), nw, False, True))
                if last_pair:
                    # merged singles: all four pairs' single blocks in one
                    # K=128 matmul each ([128][4][128] moving operand)
                    ops.append((32, pairs[:, :, 128:256, 1], 512,
                                False, False))
                    ops.append((33, pairs[:, :, 4096:4224, 0], 512,
                                True, False))
                return ops

            pending_cs = []

            def drain_cs(k):
                for _ in range(min(k, len(pending_cs))):
                    slot, rhs, nw, last, dr = pending_cs.pop(0)
                    cs_matmul(slot, rhs, nw, last=last, dr=dr)

            for i in range(4):           # row pairs
                for r in (0, 1):         # row in pair
                    m = 2 * i + r
                    base = 128 * m       # local col base of this row strip
                    off = 0
                    for p, W in enumerate(PW):
                        P = pspool.tile([128, 1536], F32, tag="ps", name="P",
                                        bufs=2)
                        for k in range(W // 512):
                            c0 = base + off + k * 512
                            nc.tensor.matmul(
                                P[:, k * 512:(k + 1) * 512],
                                xw_sb[:, :, base:base + 128],
                                xop(c0, 512),
                                start=True, stop=True,
                                perf_mode=DR)
                        # exp output slice in the pair buffer
                        n0 = off if r == 0 else off + 128
                        oslice = pair_sb[i][:, n0:n0 + W, 1 - r]
                        sidx = m * 3 + p
                        on_dve = ((p == 1 and m in DVE_P1)
                                  or (p == 2 and m in DVE_P2))
                        if on_dve:
                            exp_dve(P, W, oslice, sidx)
                        else:
                            nc.scalar.activation(
                                oslice, P[:, 0:W], AF.Exp,
                                accum_out=sexp_parts[:, sidx:sidx + 1])
                        off += W
                        drain_cs(2)
                        # pair 3's colsums can enter the queue one piece in
                        # (deps are then >= 1 piece old) to shorten the tail
                        if i == 3 and r == 1 and p == 0:
                            pending_cs.extend(
                                cs_ops_for_pair(3, last_pair=True))
                if i < 3:
                    pending_cs.extend(cs_ops_for_pair(i, last_pair=False))
            drain_cs(len(pending_cs))

            # ---- final outputs (row-sum parts reduced on host) ----
            cs_sb = ppool.tile([128, 512], F32, tag="cs_sb")
            nc.vector.tensor_copy(cs_sb[0:40, :], cs[0:40, :])
            nc.sync.dma_start(out_se[:], sexp_parts[:])
            nc.scalar.dma_start(out_cs[:], cs_sb[0:40, :])

    nc.finalize()
    return nc


def _consts():
    e5 = ml_dtypes.float8_e5m2
    oh = np.zeros((128, 2, 256), dtype=e5)
    oh[:, :, 128] = 1.0
    return oh


def _prep_x(z_full):
    """z_full [8192, 256] f32 -> X[p, j, t] = q[j, 128t+p] fp8e4 (and q)."""
    norms = np.maximum(np.sqrt((z_full.astype(np.float64) ** 2).sum(1)), 1e-8)
    q = (z_full * (SQRT10 / norms[:, None])).astype(ml_dtypes.float8_e4m3)
    x = np.ascontiguousarray(q.T.reshape(2, 128, N).transpose(1, 2, 0))
    return x, q.astype(np.float32)


def _prep_xw(xc):
    """interleaved [128, XCOLS, 2] -> m-contiguous [128, 2, 1024] head."""
    return np.ascontiguousarray(xc[:, :1024, :].transpose(0, 2, 1))


_NC_CACHE = {}


def run_device(z_full, trace=False, trace_kwargs=None):
    """z_full: [8192, 256] f32. Returns (loss_vec [8192] f32, results)."""
    if "nc" not in _NC_CACHE:
        _NC_CACHE["nc"] = build_program()
    nc = _NC_CACHE["nc"]
    oh = _consts()
    xfull, qf = _prep_x(z_full)
    in_maps = []
    for c in range(NCORES):
        xc = np.ascontiguousarray(
            np.roll(xfull, -c * ROWS, axis=1)[:, :XCOLS, :])
        in_maps.append({"xq": xc, "xw": _prep_xw(xc), "oh": oh})
    kw = {}
    if trace:
        kw["trace"] = True
        if trace_kwargs:
            kw.update(trace_kwargs)
    res = run_bass_kernel_spmd(nc, in_maps, list(range(NCORES)), **kw)

    # ring (d=32) blocks + positives, computed on host from the same q:
    # ring[b, i, j] = q[128b+i] . q[128((b+32)%64)+j]
    q3 = qf.reshape(64, 128, D)
    ring = np.einsum("bik,bjk->bij", q3, np.roll(q3, -32, axis=0),
                     optimize=True)
    posv = np.ascontiguousarray(
        np.diagonal(ring, axis1=1, axis2=2)).reshape(-1).astype(np.float32)
    rowsum = np.exp(ring.astype(np.float64)).sum(2).reshape(-1)
    # subtract the unmasked self-similarity exp(||q_i||^2) the device
    # accumulated into its row sums
    rowsum -= np.exp((qf.astype(np.float64) ** 2).sum(1))

    for c in range(NCORES):
        se = np.asarray(res.results[c]["sumexp_own"], dtype=np.float32)
        se = se.reshape(128, MB, 3).sum(2)
        csv = np.asarray(res.results[c]["colsums"], dtype=np.float32)
        g0 = c * ROWS
        rowsum[g0:g0 + ROWS] += se.T.reshape(-1).astype(np.float64)
        # colsum slots -> global rows (cols of the slab)
        for i in range(4):
            for j in range(8):
                n0 = 256 + 512 * j
                nw = min(512, 4096 - n0)
                gc = (np.arange(256 * i + n0, 256 * i + n0 + nw) + g0) % N
                np.add.at(rowsum, gc, csv[i * 8 + j, :nw].astype(np.float64))
            gc = (np.arange(256 * i + 128, 256 * i + 256) + g0) % N
            np.add.at(rowsum, gc,
                      csv[32, 128 * i:128 * i + 128].astype(np.float64))
            gc = (np.arange(256 * i + 4096, 256 * i + 4224) + g0) % N
            np.add.at(rowsum, gc,
                      csv[33, 128 * i:128 * i + 128].astype(np.float64))
    loss_vec = (np.log(rowsum) - posv).astype(np.float32)
    return loss_vec, res


def kernel(z_i, z_j, mask_positive):
    z_i = np.asarray(z_i, dtype=np.float32)
    z_j = np.asarray(z_j, dtype=np.float32)
    mask_positive = np.asarray(mask_positive)
    z_full = np.concatenate([z_i, z_j], axis=0)
    loss_vec, _ = run_device(z_full)
    if not np.isfinite(loss_vec).all():
        # rare device flake -> retry once
        loss_vec, _ = run_device(z_full)
    mp = np.concatenate([mask_positive, mask_positive]).astype(bool)
    cnt = np.float32(mp.sum())
    total = np.float32(loss_vec[mp].sum(dtype=np.float64))
    if cnt > 0:
        loss = total / np.maximum(cnt, np.float32(1.0))
    else:
        loss = np.float32(0.0)
    return np.array(loss, dtype=np.float32)
